# revision 47
# baseline (speedup 1.0000x reference)
"""Trainium2 Bass kernel for DBFLinear:
    y = ((x * s0) @ unpack(bp1).T * s2) @ unpack(bp3).T * s4 + bias

Strategy: data-parallel over batch across 8 cores (weights replicated, no
collectives). Per core: unpack the bit-packed +/-1 weights on device
(DVE bitwise_and + ACT Sign + DVE s0-fold for W1), transpose weight
blocks with the DMA xbar, run both GEMMs weight-stationary (fp16, fp32 PSUM
accumulation). scaling2 folds into the h eviction, scaling4+bias into the y
eviction (per-partition ACT ops).

DMA discipline: Tile globally serializes every copy<->transpose xbar-mode
transition (HW hang workaround), so copies are batched: weight bytes load 8
blocks per DMA, GEMM2 outputs stage into the dead xH SBUF and store in
groups, and all transposes stay on the sync queue. x transposes in 4
k-bands per batch half so the first GEMM1 pass can chase the DMA.

The device emits y.T per batch shard; the host transposes while unsharding.
"""

import sys

import numpy as np

sys.path.insert(0, "/opt/trn_rl_repo")

import concourse.bass as bass
import concourse.mybir as mybir
import concourse.tile as tile
from concourse import bacc
from concourse.bass_utils import run_bass_kernel_spmd

N_CORES = 8
B_FULL, IN, MID, OUT = 8192, 4096, 4096, 4096
P = 128
FD = 512  # matmul moving-operand free dim (1 PSUM bank of fp32)
QCH = 1024  # unpack quarter width (weight elements per DVE/ACT op)
N_WARM = 260  # HAM warm-up matmuls (drain before the first real MM is ready)
GB = 8  # weight blocks per byte-load DMA (batched to avoid xbar-mode flips)


def build_program(b=B_FULL // N_CORES, in_=IN, mid=MID, out=OUT):
    """Build the per-core Bass program. Returns the Bass object."""
    in_k, mid_k, out_k = in_ // P, mid // P, out // P
    nbc = 2  # batch processed as two halves
    fd = b // nbc
    assert fd <= FD, (b, fd)
    uch = min(QCH, in_, mid)

    nc = bacc.Bacc(num_devices=N_CORES)
    x_d = nc.dram_tensor("x", [b, in_], mybir.dt.float16, kind="ExternalInput")
    bp1_d = nc.dram_tensor("bp1", [mid, in_ // 8], mybir.dt.uint8, kind="ExternalInput")
    bp3_d = nc.dram_tensor("bp3", [out, mid // 8], mybir.dt.uint8, kind="ExternalInput")
    s0r_d = nc.dram_tensor("s0rep", [P, in_], mybir.dt.float16, kind="ExternalInput")
    s2_d = nc.dram_tensor("s2", [P, mid_k], mybir.dt.float32, kind="ExternalInput")
    s4_d = nc.dram_tensor("s4", [P, out_k], mybir.dt.float32, kind="ExternalInput")
    bias_d = nc.dram_tensor("bias", [P, out_k], mybir.dt.float32, kind="ExternalInput")
    yT_d = nc.dram_tensor("yT", [out, b], mybir.dt.float16, kind="ExternalOutput")

    Act = mybir.ActivationFunctionType

    with tile.TileContext(nc) as tc:
        with (
            tc.tile_pool(name="big", bufs=1) as big,
            tc.tile_pool(name="consts", bufs=1) as consts,
            tc.tile_pool(name="wpipe", bufs=2) as wpipe,
            tc.tile_pool(name="psum", bufs=4, space="PSUM") as psum,
        ):
            mask_t = consts.tile([P, 8], mybir.dt.uint8)
            s0r_t = consts.tile([P, in_], mybir.dt.float16)
            s2_t = consts.tile([P, mid_k], mybir.dt.float32)
            s4_t = consts.tile([P, out_k], mybir.dt.float32)
            bias_t = consts.tile([P, out_k], mybir.dt.float32)
            neg_half = consts.tile([P, 1], mybir.dt.float32)
            junk_w = consts.tile([P, 16], mybir.dt.float16)
            # mask built on-device (memsets) to keep the startup copy group
            # minimal; every copy<->transpose mode flip serializes the DMA
            # stream globally.
            for j in range(8):
                nc.vector.memset(mask_t[:, j : j + 1], 1 << (7 - j))
            nc.vector.memset(neg_half[:], -0.5)
            nc.vector.memset(junk_w[:], 0.25)

            # Warm the PE HAM clock gate with cheap junk matmuls while the
            # input pipeline fills; sized to drain just before the first real
            # matmul's operands land.
            warm_ps = psum.tile([P, 16], mybir.dt.float32, tag="warm")

            def warm_fill(n):
                for _ in range(n):
                    nc.tensor.matmul(
                        warm_ps[:16, :], junk_w[:], junk_w[:],
                        start=True, stop=True, skip_group_check=True,
                    )

            warm_fill(N_WARM)

            # Batched weight-byte loads: GB blocks per DMA so the copy/
            # transpose xbar-mode serialization fires once per group instead
            # of once per block.
            kb_bytes = {True: in_ // 8, False: mid // 8}

            def load_group(bp_d, g, k_blocks):
                kbb = k_blocks * P // 8
                byt = wpipe.tile(
                    [P, GB, kbb], mybir.dt.uint8, tag="bytes", bufs=2, name=f"byt{g}"
                )
                nc.gpsimd.dma_start(
                    byt[:],
                    bp_d[g * GB * P : (g + 1) * GB * P, :].rearrange(
                        "(j p) c -> p j c", p=P
                    ),
                )
                return byt

            def unpack_wT(byt, jg, m, k_blocks, scale_s0):
                """Unpack 128-row weight block m (slot jg of its byte-group
                tile) into its transposed [P, k_blocks, P] fp16 tile."""
                wT = wpipe.tile(
                    [P, k_blocks, P], mybir.dt.float16, tag="wT", bufs=5, name=f"wT{m}"
                )
                for c0 in range(0, k_blocks * P, uch):
                    nb = uch // 8
                    b0 = c0 // 8
                    masked = wpipe.tile([P, uch], mybir.dt.uint8, tag="masked", bufs=3)
                    in0 = byt[:, jg, b0 : b0 + nb][:, :, None].broadcast_to([P, nb, 8])
                    in1 = mask_t[:][:, None, :].broadcast_to([P, nb, 8])
                    nc.vector.tensor_tensor(
                        masked[:].rearrange("p (b j) -> p b j", j=8),
                        in0,
                        in1,
                        mybir.AluOpType.bitwise_and,
                    )
                    wq = wpipe.tile([P, uch], mybir.dt.float16, tag="wnat", bufs=4)
                    nc.scalar.activation(
                        wq[:], masked[:], Act.Sign, bias=neg_half[:, 0:1]
                    )
                    if scale_s0:
                        # block 0 stays all-DVE (shortest startup chain); later
                        # blocks split quarters across GPSIMD/DVE so their
                        # unpack chains shorten in parallel
                        eng = (
                            nc.vector
                            if m == 0 or (c0 // uch) % 2 == 1
                            else nc.gpsimd
                        )
                        eng.tensor_tensor(
                            wq[:], wq[:], s0r_t[:, c0 : c0 + uch],
                            mybir.AluOpType.mult,
                        )
                    nc.sync.dma_start_transpose(
                        wT[:, c0 // P : (c0 + uch) // P, :], wq[:]
                    )
                return wT

            # x.T in two batch halves, 4 k-bands each so the GEMM1 k-loop can
            # chase the DMA. All transposes share the sync queue (xbar mode).
            xH = [
                big.tile([P, in_k, fd], mybir.dt.float16, tag=f"xT{h}", name=f"xh{h}")
                for h in range(2)
            ]
            XB = 4
            kb = in_k // XB

            def x_band(h, band):
                nc.sync.dma_start_transpose(
                    xH[h][:, band * kb : (band + 1) * kb, :],
                    x_d[h * fd : (h + 1) * fd, band * kb * P : (band + 1) * kb * P],
                )

            hT = big.tile([P, mid_k, b], mybir.dt.float16, name="hT")

            def g1_pass(m, wT, c):
                ps = psum.tile([P, fd], mybir.dt.float32, tag="ps")
                for k in range(in_k):
                    nc.tensor.matmul(
                        ps[:],
                        wT[:, k, :],
                        xH[c][:, k, :],
                        start=(k == 0),
                        stop=(k == in_k - 1),
                    )

                nc.scalar.activation(
                    hT[:, m, c * fd : (c + 1) * fd],
                    ps[:],
                    Act.Copy,
                    scale=s2_t[:, m : m + 1],
                )

            # GEMM2 output staging: groups of blocks buffered in the (dead)
            # x-half SBUF slots, stored with one DMA per group; the final
            # group is kept small so the tail store is short.
            yT_v = yT_d.rearrange("(g p) c -> p g c", p=P)
            ygroups = []
            _o = 0
            while _o < out_k:
                rem = out_k - _o
                if rem > 8:
                    n = 8
                elif rem > 2:
                    n = rem - 2
                else:
                    n = 1
                ygroups.append((_o, n))
                _o += n
            o2group = {}
            for gi_, (gs, gn) in enumerate(ygroups):
                for oo in range(gs, gs + gn):
                    o2group[oo] = (gi_, gs, gn)
            yt_state = [None]

            def g2_pass(o, wT):
                gi_, gstart, glen = o2group[o]
                if o == gstart:
                    yt_state[0] = big.tile(
                        [P, glen, b], mybir.dt.float16,
                        tag=f"xT{gi_ % 2}", name=f"ytg{o}",
                    )
                yt_g = yt_state[0]
                for c in range(nbc):
                    ps = psum.tile([P, fd], mybir.dt.float32, tag="ps")
                    for k in range(mid_k):
                        nc.tensor.matmul(
                            ps[:],
                            wT[:, k, :],
                            hT[:, k, c * fd : (c + 1) * fd],
                            start=(k == 0),
                            stop=(k == mid_k - 1),
                        )
                    nc.scalar.activation(
                        yt_g[:, o - gstart, c * fd : (c + 1) * fd],
                        ps[:],
                        Act.Identity,
                        bias=bias_t[:, o : o + 1],
                        scale=s4_t[:, o : o + 1],
                    )
                if o == gstart + glen - 1:
                    nc.gpsimd.dma_start(yT_v[:, gstart : gstart + glen, :], yt_g[:])

            # Startup: first byte group + x half 0, unpack the first SB
            # blocks, x half 1; then the unified loop with unpack prefetched
            # SB blocks ahead (new byte groups loaded one group ahead).
            SB = 4
            n_blocks = mid_k + out_k
            n_groups = n_blocks // GB
            bgroups = [None] * n_groups

            def group_of(jj):
                g = jj // GB
                if bgroups[g] is None:
                    if g < mid_k // GB:
                        bgroups[g] = load_group(bp1_d, g, in_k)
                    else:
                        bgroups[g] = load_group(bp3_d, g - mid_k // GB, mid_k)
                return bgroups[g]

            def mk(jj):
                byt = group_of(jj)
                if jj + GB < n_blocks and (jj + 2) % GB == 0:
                    group_of(jj + GB)  # prefetch next byte group early
                if jj < mid_k:
                    return unpack_wT(byt, jj % GB, jj, in_k, True)
                return unpack_wT(byt, jj % GB, jj, mid_k, False)

            # Startup: byte group 0 on gpsimd; consts copies lead the sync
            # queue (before its transposes — the copy<->transpose xbar-mode
            # serialization then fires once); x bands interleave with the
            # first blocks' unpack transposes so neither hogs the queue.
            group_of(0)
            for t, dd in (
                (s0r_t, s0r_d),
                (s2_t, s2_d),
                (s4_t, s4_d),
                (bias_t, bias_d),
            ):
                nc.sync.dma_start(t[:], dd[:])
            x_band(0, 0)
            pend = [mk(0)]
            for band in range(1, XB):
                x_band(0, band)
            pend.append(mk(1))
            x_band(1, 0)
            x_band(1, 1)
            pend.append(mk(2))
            x_band(1, 2)
            x_band(1, 3)
            pend.append(mk(3))

            for j in range(n_blocks):
                wT = pend.pop(0)
                if j < mid_k:
                    for c in range(nbc):
                        g1_pass(j, wT, c)
                else:
                    g2_pass(j - mid_k, wT)
                if j + SB < n_blocks:
                    pend.append(mk(j + SB))

    nc.compile()
    return nc


def make_in_maps(x, scaling0, bp1, scaling2, bp3, scaling4, bias, n_cores=N_CORES):
    b_full, in_ = x.shape
    mid = scaling2.shape[0]
    out = scaling4.shape[0]
    b = b_full // n_cores

    def pcol(v):
        return np.ascontiguousarray(v.astype(np.float32).reshape(-1, P).T)

    shared = {
        "bp1": np.ascontiguousarray(bp1.reshape(mid, in_ // 8).astype(np.uint8)),
        "bp3": np.ascontiguousarray(bp3.reshape(out, mid // 8).astype(np.uint8)),
        "s0rep": np.ascontiguousarray(
            np.broadcast_to(scaling0.astype(np.float16)[None, :], (P, in_))
        ),
        "s2": pcol(scaling2),
        "s4": pcol(scaling4),
        "bias": pcol(bias),
    }
    return [
        {"x": np.ascontiguousarray(x[c * b : (c + 1) * b]), **shared}
        for c in range(n_cores)
    ]


_PROGRAM_CACHE = {}


def run(x, scaling0, bp1, scaling2, bp3, scaling4, bias, **spmd_kwargs):
    """Compile (cached) + run on 8 cores; returns (y, BassKernelResults)."""
    if "nc" not in _PROGRAM_CACHE:
        _PROGRAM_CACHE["nc"] = build_program()
    nc = _PROGRAM_CACHE["nc"]
    in_maps = make_in_maps(x, scaling0, bp1, scaling2, bp3, scaling4, bias)
    res = run_bass_kernel_spmd(nc, in_maps, core_ids=list(range(N_CORES)), **spmd_kwargs)
    b = x.shape[0] // N_CORES
    y = np.empty((x.shape[0], scaling4.shape[0]), dtype=np.float16)
    for c in range(N_CORES):
        y[c * b : (c + 1) * b] = res.results[c]["yT"].T
    return y, res


def kernel(x, scaling0, bp1, scaling2, bp3, scaling4, bias):
    y, _ = run(x, scaling0, bp1, scaling2, bp3, scaling4, bias)
    return y


# revision 48
# speedup vs baseline: 1.1251x; 1.1251x over previous
"""Trainium2 Bass kernel for DBFLinear:
    y = ((x * s0) @ unpack(bp1).T * s2) @ unpack(bp3).T * s4 + bias

Strategy: data-parallel over batch across 8 cores (weights replicated, no
collectives). Per core: unpack the bit-packed +/-1 weights on device
(DVE bitwise_and + ACT Sign + DVE s0-fold for W1), transpose weight
blocks with the DMA xbar, run both GEMMs weight-stationary (fp16, fp32 PSUM
accumulation). scaling2 folds into the h eviction, scaling4+bias into the y
eviction (per-partition ACT ops).

DMA discipline: Tile globally serializes every copy<->transpose xbar-mode
transition (HW hang workaround), so copies are batched: weight bytes load 8
blocks per DMA, GEMM2 outputs stage into the dead xH SBUF and store in
groups, and all transposes stay on the sync queue. x transposes in 4
k-bands per batch half so the first GEMM1 pass can chase the DMA.

The device emits y.T per batch shard; the host transposes while unsharding.
"""

import sys

import numpy as np

sys.path.insert(0, "/opt/trn_rl_repo")

import concourse.bass as bass
import concourse.mybir as mybir
import concourse.tile as tile
from concourse import bacc
from concourse.bass_utils import run_bass_kernel_spmd

N_CORES = 8
B_FULL, IN, MID, OUT = 8192, 4096, 4096, 4096
P = 128
FD = 512  # matmul moving-operand free dim (1 PSUM bank of fp32)
QCH = 1024  # unpack quarter width (weight elements per DVE/ACT op)
N_WARM = 260  # HAM warm-up matmuls (drain before the first real MM is ready)
GB = 8  # weight blocks per byte-load DMA (batched to avoid xbar-mode flips)


def build_program(b=B_FULL // N_CORES, in_=IN, mid=MID, out=OUT):
    """Build the per-core Bass program. Returns the Bass object."""
    in_k, mid_k, out_k = in_ // P, mid // P, out // P
    nbc = 2  # batch processed as two halves
    fd = b // nbc
    assert fd <= FD, (b, fd)
    uch = min(QCH, in_, mid)

    nc = bacc.Bacc(num_devices=N_CORES)
    x_d = nc.dram_tensor("x", [b, in_], mybir.dt.float16, kind="ExternalInput")
    bp1_d = nc.dram_tensor("bp1", [mid, in_ // 8], mybir.dt.uint8, kind="ExternalInput")
    bp3_d = nc.dram_tensor("bp3", [out, mid // 8], mybir.dt.uint8, kind="ExternalInput")
    s0r_d = nc.dram_tensor("s0rep", [P, in_], mybir.dt.float16, kind="ExternalInput")
    s2_d = nc.dram_tensor("s2", [P, mid_k], mybir.dt.float32, kind="ExternalInput")
    s4_d = nc.dram_tensor("s4", [P, out_k], mybir.dt.float32, kind="ExternalInput")
    bias_d = nc.dram_tensor("bias", [P, out_k], mybir.dt.float32, kind="ExternalInput")
    yT_d = nc.dram_tensor("yT", [out, b], mybir.dt.float16, kind="ExternalOutput")

    Act = mybir.ActivationFunctionType

    with tile.TileContext(nc) as tc:
        with (
            tc.tile_pool(name="big", bufs=1) as big,
            tc.tile_pool(name="consts", bufs=1) as consts,
            tc.tile_pool(name="wpipe", bufs=2) as wpipe,
            tc.tile_pool(name="psum", bufs=4, space="PSUM") as psum,
        ):
            mask_t = consts.tile([P, 8], mybir.dt.uint8)
            s0r_t = consts.tile([P, in_], mybir.dt.float16)
            s2_t = consts.tile([P, mid_k], mybir.dt.float32)
            s4_t = consts.tile([P, out_k], mybir.dt.float32)
            bias_t = consts.tile([P, out_k], mybir.dt.float32)
            neg_half = consts.tile([P, 1], mybir.dt.float32)
            junk_w = consts.tile([P, 16], mybir.dt.float16)
            # mask built on-device (memsets) to keep the startup copy group
            # minimal; every copy<->transpose mode flip serializes the DMA
            # stream globally.
            for j in range(8):
                nc.vector.memset(mask_t[:, j : j + 1], 1 << (7 - j))
            nc.vector.memset(neg_half[:], -0.5)
            nc.vector.memset(junk_w[:], 0.25)

            # Warm the PE HAM clock gate with cheap junk matmuls while the
            # input pipeline fills; sized to drain just before the first real
            # matmul's operands land.
            warm_ps = psum.tile([P, 16], mybir.dt.float32, tag="warm")

            def warm_fill(n):
                for _ in range(n):
                    nc.tensor.matmul(
                        warm_ps[:16, :], junk_w[:], junk_w[:],
                        start=True, stop=True, skip_group_check=True,
                    )

            warm_fill(N_WARM)

            # Batched weight-byte loads: GB blocks per DMA so the copy/
            # transpose xbar-mode serialization fires once per group instead
            # of once per block.
            kb_bytes = {True: in_ // 8, False: mid // 8}

            def load_group(bp_d, g, k_blocks):
                kbb = k_blocks * P // 8
                byt = wpipe.tile(
                    [P, GB, kbb], mybir.dt.uint8, tag="bytes", bufs=2, name=f"byt{g}"
                )
                nc.gpsimd.dma_start(
                    byt[:],
                    bp_d[g * GB * P : (g + 1) * GB * P, :].rearrange(
                        "(j p) c -> p j c", p=P
                    ),
                )
                return byt

            def unpack_wT(byt, jg, m, k_blocks, scale_s0):
                """Unpack 128-row weight block m (slot jg of its byte-group
                tile) into its transposed [P, k_blocks, P] fp16 tile."""
                wT = wpipe.tile(
                    [P, k_blocks, P], mybir.dt.float16, tag="wT", bufs=5, name=f"wT{m}"
                )
                for c0 in range(0, k_blocks * P, uch):
                    nb = uch // 8
                    b0 = c0 // 8
                    masked = wpipe.tile([P, uch], mybir.dt.uint8, tag="masked", bufs=3)
                    in0 = byt[:, jg, b0 : b0 + nb][:, :, None].broadcast_to([P, nb, 8])
                    in1 = mask_t[:][:, None, :].broadcast_to([P, nb, 8])
                    nc.vector.tensor_tensor(
                        masked[:].rearrange("p (b j) -> p b j", j=8),
                        in0,
                        in1,
                        mybir.AluOpType.bitwise_and,
                    )
                    wq = wpipe.tile([P, uch], mybir.dt.float16, tag="wnat", bufs=4)
                    nc.scalar.activation(
                        wq[:], masked[:], Act.Sign, bias=neg_half[:, 0:1]
                    )
                    if scale_s0:
                        # DVE runs this at 2x (packed fp16) and its queue has
                        # no DMA copies to serialize behind, unlike GPSIMD
                        nc.vector.tensor_tensor(
                            wq[:], wq[:], s0r_t[:, c0 : c0 + uch],
                            mybir.AluOpType.mult,
                        )
                    nc.sync.dma_start_transpose(
                        wT[:, c0 // P : (c0 + uch) // P, :], wq[:]
                    )
                return wT

            # x.T in two batch halves, 4 k-bands each so the GEMM1 k-loop can
            # chase the DMA. All transposes share the sync queue (xbar mode).
            xH = [
                big.tile([P, in_k, fd], mybir.dt.float16, tag=f"xT{h}", name=f"xh{h}")
                for h in range(2)
            ]
            XB = 4
            kb = in_k // XB

            def x_band(h, band):
                nc.sync.dma_start_transpose(
                    xH[h][:, band * kb : (band + 1) * kb, :],
                    x_d[h * fd : (h + 1) * fd, band * kb * P : (band + 1) * kb * P],
                )

            hT = big.tile([P, mid_k, b], mybir.dt.float16, name="hT")

            def g1_pass(m, wT, c):
                ps = psum.tile([P, fd], mybir.dt.float32, tag="ps")
                for k in range(in_k):
                    nc.tensor.matmul(
                        ps[:],
                        wT[:, k, :],
                        xH[c][:, k, :],
                        start=(k == 0),
                        stop=(k == in_k - 1),
                    )

                nc.scalar.activation(
                    hT[:, m, c * fd : (c + 1) * fd],
                    ps[:],
                    Act.Copy,
                    scale=s2_t[:, m : m + 1],
                )

            # GEMM2 output staging: groups of blocks buffered in the (dead)
            # x-half SBUF slots, stored with one DMA per group; the final
            # group is kept small so the tail store is short.
            yT_v = yT_d.rearrange("(g p) c -> p g c", p=P)
            ygroups = []
            _o = 0
            while _o < out_k:
                rem = out_k - _o
                if rem > 8:
                    n = 8
                elif rem > 2:
                    n = rem - 2
                else:
                    n = 1
                ygroups.append((_o, n))
                _o += n
            o2group = {}
            for gi_, (gs, gn) in enumerate(ygroups):
                for oo in range(gs, gs + gn):
                    o2group[oo] = (gi_, gs, gn)
            yt_state = [None]

            def g2_pass(o, wT):
                gi_, gstart, glen = o2group[o]
                if o == gstart:
                    yt_state[0] = big.tile(
                        [P, glen, b], mybir.dt.float16,
                        tag=f"xT{gi_ % 2}", name=f"ytg{o}",
                    )
                yt_g = yt_state[0]
                for c in range(nbc):
                    ps = psum.tile([P, fd], mybir.dt.float32, tag="ps")
                    for k in range(mid_k):
                        nc.tensor.matmul(
                            ps[:],
                            wT[:, k, :],
                            hT[:, k, c * fd : (c + 1) * fd],
                            start=(k == 0),
                            stop=(k == mid_k - 1),
                        )
                    nc.scalar.activation(
                        yt_g[:, o - gstart, c * fd : (c + 1) * fd],
                        ps[:],
                        Act.Identity,
                        bias=bias_t[:, o : o + 1],
                        scale=s4_t[:, o : o + 1],
                    )
                if o == gstart + glen - 1:
                    nc.gpsimd.dma_start(yT_v[:, gstart : gstart + glen, :], yt_g[:])

            # Startup: first byte group + x half 0, unpack the first SB
            # blocks, x half 1; then the unified loop with unpack prefetched
            # SB blocks ahead (new byte groups loaded one group ahead).
            SB = 4
            n_blocks = mid_k + out_k
            n_groups = n_blocks // GB
            bgroups = [None] * n_groups

            def group_of(jj):
                g = jj // GB
                if bgroups[g] is None:
                    if g < mid_k // GB:
                        bgroups[g] = load_group(bp1_d, g, in_k)
                    else:
                        bgroups[g] = load_group(bp3_d, g - mid_k // GB, mid_k)
                return bgroups[g]

            def mk(jj):
                byt = group_of(jj)
                if jj + GB < n_blocks and (jj + 2) % GB == 0:
                    group_of(jj + GB)  # prefetch next byte group early
                if jj < mid_k:
                    return unpack_wT(byt, jj % GB, jj, in_k, True)
                return unpack_wT(byt, jj % GB, jj, mid_k, False)

            # Startup: byte group 0 on gpsimd; consts copies lead the sync
            # queue (before its transposes — the copy<->transpose xbar-mode
            # serialization then fires once); x bands interleave with the
            # first blocks' unpack transposes so neither hogs the queue.
            group_of(0)
            for t, dd in (
                (s0r_t, s0r_d),
                (s2_t, s2_d),
                (s4_t, s4_d),
                (bias_t, bias_d),
            ):
                nc.sync.dma_start(t[:], dd[:])
            x_band(0, 0)
            pend = [mk(0)]
            for band in range(1, XB):
                x_band(0, band)
            pend.append(mk(1))
            x_band(1, 0)
            x_band(1, 1)
            pend.append(mk(2))
            x_band(1, 2)
            x_band(1, 3)
            pend.append(mk(3))

            for j in range(n_blocks):
                wT = pend.pop(0)
                if j < mid_k:
                    for c in range(nbc):
                        g1_pass(j, wT, c)
                else:
                    g2_pass(j - mid_k, wT)
                if j + SB < n_blocks:
                    pend.append(mk(j + SB))

    nc.compile()
    return nc


def make_in_maps(x, scaling0, bp1, scaling2, bp3, scaling4, bias, n_cores=N_CORES):
    b_full, in_ = x.shape
    mid = scaling2.shape[0]
    out = scaling4.shape[0]
    b = b_full // n_cores

    def pcol(v):
        return np.ascontiguousarray(v.astype(np.float32).reshape(-1, P).T)

    shared = {
        "bp1": np.ascontiguousarray(bp1.reshape(mid, in_ // 8).astype(np.uint8)),
        "bp3": np.ascontiguousarray(bp3.reshape(out, mid // 8).astype(np.uint8)),
        "s0rep": np.ascontiguousarray(
            np.broadcast_to(scaling0.astype(np.float16)[None, :], (P, in_))
        ),
        "s2": pcol(scaling2),
        "s4": pcol(scaling4),
        "bias": pcol(bias),
    }
    return [
        {"x": np.ascontiguousarray(x[c * b : (c + 1) * b]), **shared}
        for c in range(n_cores)
    ]


_PROGRAM_CACHE = {}


def run(x, scaling0, bp1, scaling2, bp3, scaling4, bias, **spmd_kwargs):
    """Compile (cached) + run on 8 cores; returns (y, BassKernelResults)."""
    if "nc" not in _PROGRAM_CACHE:
        _PROGRAM_CACHE["nc"] = build_program()
    nc = _PROGRAM_CACHE["nc"]
    in_maps = make_in_maps(x, scaling0, bp1, scaling2, bp3, scaling4, bias)
    res = run_bass_kernel_spmd(nc, in_maps, core_ids=list(range(N_CORES)), **spmd_kwargs)
    b = x.shape[0] // N_CORES
    y = np.empty((x.shape[0], scaling4.shape[0]), dtype=np.float16)
    for c in range(N_CORES):
        y[c * b : (c + 1) * b] = res.results[c]["yT"].T
    return y, res


def kernel(x, scaling0, bp1, scaling2, bp3, scaling4, bias):
    y, _ = run(x, scaling0, bp1, scaling2, bp3, scaling4, bias)
    return y


# revision 51
# speedup vs baseline: 1.1256x; 1.0004x over previous
"""Trainium2 Bass kernel for DBFLinear:
    y = ((x * s0) @ unpack(bp1).T * s2) @ unpack(bp3).T * s4 + bias

Strategy: data-parallel over batch across 8 cores (weights replicated, no
collectives). Per core: unpack the bit-packed +/-1 weights on device
(DVE bitwise_and + ACT Sign + DVE s0-fold for W1), transpose weight
blocks with the DMA xbar, run both GEMMs weight-stationary (fp16, fp32 PSUM
accumulation). scaling2 folds into the h eviction, scaling4+bias into the y
eviction (per-partition ACT ops).

DMA discipline: Tile globally serializes every copy<->transpose xbar-mode
transition (HW hang workaround), so copies are batched: weight bytes load 8
blocks per DMA, GEMM2 outputs stage into the dead xH SBUF and store in
groups, and all transposes stay on the sync queue. x transposes in 4
k-bands per batch half so the first GEMM1 pass can chase the DMA.

The device emits y.T per batch shard; the host transposes while unsharding.
"""

import sys

import numpy as np

sys.path.insert(0, "/opt/trn_rl_repo")

import concourse.bass as bass
import concourse.mybir as mybir
import concourse.tile as tile
from concourse import bacc
from concourse.bass_utils import run_bass_kernel_spmd

N_CORES = 8
B_FULL, IN, MID, OUT = 8192, 4096, 4096, 4096
P = 128
FD = 512  # matmul moving-operand free dim (1 PSUM bank of fp32)
QCH = 2048  # unpack chunk width (weight elements per DVE/ACT op)
N_WARM = 260  # HAM warm-up matmuls (drain before the first real MM is ready)
GB = 8  # weight blocks per byte-load DMA (batched to avoid xbar-mode flips)


def build_program(b=B_FULL // N_CORES, in_=IN, mid=MID, out=OUT):
    """Build the per-core Bass program. Returns the Bass object."""
    in_k, mid_k, out_k = in_ // P, mid // P, out // P
    nbc = 2  # batch processed as two halves
    fd = b // nbc
    assert fd <= FD, (b, fd)
    uch = min(QCH, in_, mid)

    nc = bacc.Bacc(num_devices=N_CORES)
    x_d = nc.dram_tensor("x", [b, in_], mybir.dt.float16, kind="ExternalInput")
    bp1_d = nc.dram_tensor("bp1", [mid, in_ // 8], mybir.dt.uint8, kind="ExternalInput")
    bp3_d = nc.dram_tensor("bp3", [out, mid // 8], mybir.dt.uint8, kind="ExternalInput")
    s0r_d = nc.dram_tensor("s0rep", [P, in_], mybir.dt.float16, kind="ExternalInput")
    s2_d = nc.dram_tensor("s2", [P, mid_k], mybir.dt.float32, kind="ExternalInput")
    s4_d = nc.dram_tensor("s4", [P, out_k], mybir.dt.float32, kind="ExternalInput")
    bias_d = nc.dram_tensor("bias", [P, out_k], mybir.dt.float32, kind="ExternalInput")
    yT_d = nc.dram_tensor("yT", [out, b], mybir.dt.float16, kind="ExternalOutput")

    Act = mybir.ActivationFunctionType

    with tile.TileContext(nc) as tc:
        with (
            tc.tile_pool(name="big", bufs=1) as big,
            tc.tile_pool(name="consts", bufs=1) as consts,
            tc.tile_pool(name="wpipe", bufs=2) as wpipe,
            tc.tile_pool(name="psum", bufs=4, space="PSUM") as psum,
        ):
            mask_t = consts.tile([P, 8], mybir.dt.uint8)
            s0r_t = consts.tile([P, in_], mybir.dt.float16)
            s2_t = consts.tile([P, mid_k], mybir.dt.float32)
            s4_t = consts.tile([P, out_k], mybir.dt.float32)
            bias_t = consts.tile([P, out_k], mybir.dt.float32)
            neg_half = consts.tile([P, 1], mybir.dt.float32)
            junk_w = consts.tile([P, 16], mybir.dt.float16)
            # mask built on-device (memsets) to keep the startup copy group
            # minimal; every copy<->transpose mode flip serializes the DMA
            # stream globally.
            for j in range(8):
                nc.vector.memset(mask_t[:, j : j + 1], 1 << (7 - j))
            nc.vector.memset(neg_half[:], -0.5)
            nc.vector.memset(junk_w[:], 0.25)

            # Warm the PE HAM clock gate with cheap junk matmuls while the
            # input pipeline fills; sized to drain just before the first real
            # matmul's operands land.
            warm_ps = psum.tile([P, 16], mybir.dt.float32, tag="warm")

            def warm_fill(n):
                for _ in range(n):
                    nc.tensor.matmul(
                        warm_ps[:16, :], junk_w[:], junk_w[:],
                        start=True, stop=True, skip_group_check=True,
                    )

            warm_fill(N_WARM)

            # Batched weight-byte loads: GB blocks per DMA so the copy/
            # transpose xbar-mode serialization fires once per group instead
            # of once per block.
            kb_bytes = {True: in_ // 8, False: mid // 8}

            def load_group(bp_d, g, k_blocks):
                kbb = k_blocks * P // 8
                byt = wpipe.tile(
                    [P, GB, kbb], mybir.dt.uint8, tag="bytes", bufs=2, name=f"byt{g}"
                )
                nc.gpsimd.dma_start(
                    byt[:],
                    bp_d[g * GB * P : (g + 1) * GB * P, :].rearrange(
                        "(j p) c -> p j c", p=P
                    ),
                )
                return byt

            def unpack_wT(byt, jg, m, k_blocks, scale_s0):
                """Unpack 128-row weight block m (slot jg of its byte-group
                tile) into its transposed [P, k_blocks, P] fp16 tile."""
                wT = wpipe.tile(
                    [P, k_blocks, P], mybir.dt.float16, tag="wT", bufs=5, name=f"wT{m}"
                )
                for c0 in range(0, k_blocks * P, uch):
                    nb = uch // 8
                    b0 = c0 // 8
                    masked = wpipe.tile([P, uch], mybir.dt.uint8, tag="masked", bufs=2)
                    in0 = byt[:, jg, b0 : b0 + nb][:, :, None].broadcast_to([P, nb, 8])
                    in1 = mask_t[:][:, None, :].broadcast_to([P, nb, 8])
                    nc.vector.tensor_tensor(
                        masked[:].rearrange("p (b j) -> p b j", j=8),
                        in0,
                        in1,
                        mybir.AluOpType.bitwise_and,
                    )
                    wq = wpipe.tile([P, uch], mybir.dt.float16, tag="wnat", bufs=3)
                    nc.scalar.activation(
                        wq[:], masked[:], Act.Sign, bias=neg_half[:, 0:1]
                    )
                    if scale_s0:
                        # DVE runs this at 2x (packed fp16) and its queue has
                        # no DMA copies to serialize behind, unlike GPSIMD
                        nc.vector.tensor_tensor(
                            wq[:], wq[:], s0r_t[:, c0 : c0 + uch],
                            mybir.AluOpType.mult,
                        )
                    nc.sync.dma_start_transpose(
                        wT[:, c0 // P : (c0 + uch) // P, :], wq[:]
                    )
                return wT

            # x.T in two batch halves, 4 k-bands each so the GEMM1 k-loop can
            # chase the DMA. All transposes share the sync queue (xbar mode).
            xH = [
                big.tile([P, in_k, fd], mybir.dt.float16, tag=f"xT{h}", name=f"xh{h}")
                for h in range(2)
            ]
            XB = 4
            kb = in_k // XB

            def x_band(h, band):
                nc.sync.dma_start_transpose(
                    xH[h][:, band * kb : (band + 1) * kb, :],
                    x_d[h * fd : (h + 1) * fd, band * kb * P : (band + 1) * kb * P],
                )

            hT = big.tile([P, mid_k, b], mybir.dt.float16, name="hT")

            def g1_pass(m, wT, c):
                ps = psum.tile([P, fd], mybir.dt.float32, tag="ps")
                for k in range(in_k):
                    nc.tensor.matmul(
                        ps[:],
                        wT[:, k, :],
                        xH[c][:, k, :],
                        start=(k == 0),
                        stop=(k == in_k - 1),
                    )

                nc.scalar.activation(
                    hT[:, m, c * fd : (c + 1) * fd],
                    ps[:],
                    Act.Copy,
                    scale=s2_t[:, m : m + 1],
                )

            # GEMM2 output staging: groups of blocks buffered in the (dead)
            # x-half SBUF slots, stored with one DMA per group; the final
            # group is kept small so the tail store is short.
            yT_v = yT_d.rearrange("(g p) c -> p g c", p=P)
            ygroups = []
            _o = 0
            while _o < out_k:
                rem = out_k - _o
                if rem > 8:
                    n = 8
                elif rem > 2:
                    n = rem - 2
                else:
                    n = 1
                ygroups.append((_o, n))
                _o += n
            o2group = {}
            for gi_, (gs, gn) in enumerate(ygroups):
                for oo in range(gs, gs + gn):
                    o2group[oo] = (gi_, gs, gn)
            yt_state = [None]

            def g2_pass(o, wT):
                gi_, gstart, glen = o2group[o]
                if o == gstart:
                    yt_state[0] = big.tile(
                        [P, glen, b], mybir.dt.float16,
                        tag=f"xT{gi_ % 2}", name=f"ytg{o}",
                    )
                yt_g = yt_state[0]
                for c in range(nbc):
                    ps = psum.tile([P, fd], mybir.dt.float32, tag="ps")
                    for k in range(mid_k):
                        nc.tensor.matmul(
                            ps[:],
                            wT[:, k, :],
                            hT[:, k, c * fd : (c + 1) * fd],
                            start=(k == 0),
                            stop=(k == mid_k - 1),
                        )
                    nc.scalar.activation(
                        yt_g[:, o - gstart, c * fd : (c + 1) * fd],
                        ps[:],
                        Act.Identity,
                        bias=bias_t[:, o : o + 1],
                        scale=s4_t[:, o : o + 1],
                    )
                if o == gstart + glen - 1:
                    nc.gpsimd.dma_start(yT_v[:, gstart : gstart + glen, :], yt_g[:])

            # Startup: first byte group + x half 0, unpack the first SB
            # blocks, x half 1; then the unified loop with unpack prefetched
            # SB blocks ahead (new byte groups loaded one group ahead).
            SB = 4
            n_blocks = mid_k + out_k
            n_groups = n_blocks // GB
            bgroups = [None] * n_groups

            def group_of(jj):
                g = jj // GB
                if bgroups[g] is None:
                    if g < mid_k // GB:
                        bgroups[g] = load_group(bp1_d, g, in_k)
                    else:
                        bgroups[g] = load_group(bp3_d, g - mid_k // GB, mid_k)
                return bgroups[g]

            def mk(jj):
                byt = group_of(jj)
                if jj + GB < n_blocks and (jj + 2) % GB == 0:
                    group_of(jj + GB)  # prefetch next byte group early
                if jj < mid_k:
                    return unpack_wT(byt, jj % GB, jj, in_k, True)
                return unpack_wT(byt, jj % GB, jj, mid_k, False)

            # Startup: byte group 0 on gpsimd; consts copies lead the sync
            # queue (before its transposes — the copy<->transpose xbar-mode
            # serialization then fires once); x bands interleave with the
            # first blocks' unpack transposes so neither hogs the queue.
            group_of(0)
            for t, dd in (
                (s0r_t, s0r_d),
                (s2_t, s2_d),
                (s4_t, s4_d),
                (bias_t, bias_d),
            ):
                nc.sync.dma_start(t[:], dd[:])
            x_band(0, 0)
            pend = [mk(0)]
            for band in range(1, XB):
                x_band(0, band)
            pend.append(mk(1))
            x_band(1, 0)
            x_band(1, 1)
            pend.append(mk(2))
            x_band(1, 2)
            x_band(1, 3)
            pend.append(mk(3))

            for j in range(n_blocks):
                wT = pend.pop(0)
                if j < mid_k:
                    for c in range(nbc):
                        g1_pass(j, wT, c)
                else:
                    g2_pass(j - mid_k, wT)
                if j + SB < n_blocks:
                    pend.append(mk(j + SB))

    nc.compile()
    return nc


def make_in_maps(x, scaling0, bp1, scaling2, bp3, scaling4, bias, n_cores=N_CORES):
    b_full, in_ = x.shape
    mid = scaling2.shape[0]
    out = scaling4.shape[0]
    b = b_full // n_cores

    def pcol(v):
        return np.ascontiguousarray(v.astype(np.float32).reshape(-1, P).T)

    shared = {
        "bp1": np.ascontiguousarray(bp1.reshape(mid, in_ // 8).astype(np.uint8)),
        "bp3": np.ascontiguousarray(bp3.reshape(out, mid // 8).astype(np.uint8)),
        "s0rep": np.ascontiguousarray(
            np.broadcast_to(scaling0.astype(np.float16)[None, :], (P, in_))
        ),
        "s2": pcol(scaling2),
        "s4": pcol(scaling4),
        "bias": pcol(bias),
    }
    return [
        {"x": np.ascontiguousarray(x[c * b : (c + 1) * b]), **shared}
        for c in range(n_cores)
    ]


_PROGRAM_CACHE = {}


def run(x, scaling0, bp1, scaling2, bp3, scaling4, bias, **spmd_kwargs):
    """Compile (cached) + run on 8 cores; returns (y, BassKernelResults)."""
    if "nc" not in _PROGRAM_CACHE:
        _PROGRAM_CACHE["nc"] = build_program()
    nc = _PROGRAM_CACHE["nc"]
    in_maps = make_in_maps(x, scaling0, bp1, scaling2, bp3, scaling4, bias)
    res = run_bass_kernel_spmd(nc, in_maps, core_ids=list(range(N_CORES)), **spmd_kwargs)
    b = x.shape[0] // N_CORES
    y = np.empty((x.shape[0], scaling4.shape[0]), dtype=np.float16)
    for c in range(N_CORES):
        y[c * b : (c + 1) * b] = res.results[c]["yT"].T
    return y, res


def kernel(x, scaling0, bp1, scaling2, bp3, scaling4, bias):
    y, _ = run(x, scaling0, bp1, scaling2, bp3, scaling4, bias)
    return y


# revision 55
# speedup vs baseline: 1.1302x; 1.0041x over previous
"""Trainium2 Bass kernel for DBFLinear:
    y = ((x * s0) @ unpack(bp1).T * s2) @ unpack(bp3).T * s4 + bias

Strategy: data-parallel over batch across 8 cores (weights replicated, no
collectives). Per core: unpack the bit-packed +/-1 weights on device
(DVE bitwise_and + ACT Sign + DVE s0-fold for W1), transpose weight
blocks with the DMA xbar, run both GEMMs weight-stationary (fp16, fp32 PSUM
accumulation). scaling2 folds into the h eviction, scaling4+bias into the y
eviction (per-partition ACT ops).

DMA discipline: Tile globally serializes every copy<->transpose xbar-mode
transition (HW hang workaround), so copies are batched: weight bytes load 8
blocks per DMA, GEMM2 outputs stage into the dead xH SBUF and store in
groups, and all transposes stay on the sync queue. x transposes in 4
k-bands per batch half so the first GEMM1 pass can chase the DMA.

The device emits y.T per batch shard; the host transposes while unsharding.
"""

import sys

import numpy as np

sys.path.insert(0, "/opt/trn_rl_repo")

import concourse.bass as bass
import concourse.mybir as mybir
import concourse.tile as tile
from concourse import bacc
from concourse.bass_utils import run_bass_kernel_spmd

N_CORES = 8
B_FULL, IN, MID, OUT = 8192, 4096, 4096, 4096
P = 128
FD = 512  # matmul moving-operand free dim (1 PSUM bank of fp32)
QCH = 2048  # unpack chunk width (weight elements per DVE/ACT op)
N_WARM = 260  # HAM warm-up matmuls (drain before the first real MM is ready)
GB = 8  # weight blocks per byte-load DMA (batched to avoid xbar-mode flips)


def build_program(b=B_FULL // N_CORES, in_=IN, mid=MID, out=OUT):
    """Build the per-core Bass program. Returns the Bass object."""
    in_k, mid_k, out_k = in_ // P, mid // P, out // P
    nbc = 2  # batch processed as two halves
    fd = b // nbc
    assert fd <= FD, (b, fd)
    uch = min(QCH, in_, mid)

    nc = bacc.Bacc(num_devices=N_CORES)
    x_d = nc.dram_tensor("x", [b, in_], mybir.dt.float16, kind="ExternalInput")
    bp1_d = nc.dram_tensor("bp1", [mid, in_ // 8], mybir.dt.uint8, kind="ExternalInput")
    bp3_d = nc.dram_tensor("bp3", [out, mid // 8], mybir.dt.uint8, kind="ExternalInput")
    s0r_d = nc.dram_tensor("s0rep", [P, in_], mybir.dt.float16, kind="ExternalInput")
    s2_d = nc.dram_tensor("s2", [P, mid_k], mybir.dt.float32, kind="ExternalInput")
    s4_d = nc.dram_tensor("s4", [P, out_k], mybir.dt.float32, kind="ExternalInput")
    bias_d = nc.dram_tensor("bias", [P, out_k], mybir.dt.float32, kind="ExternalInput")
    yT_d = nc.dram_tensor("yT", [out, b], mybir.dt.float16, kind="ExternalOutput")

    Act = mybir.ActivationFunctionType

    with tile.TileContext(nc) as tc:
        with (
            tc.tile_pool(name="big", bufs=1) as big,
            tc.tile_pool(name="consts", bufs=1) as consts,
            tc.tile_pool(name="wpipe", bufs=2) as wpipe,
            tc.tile_pool(name="psum", bufs=4, space="PSUM") as psum,
        ):
            mask_t = consts.tile([P, 8], mybir.dt.uint8)
            s0r_t = consts.tile([P, in_], mybir.dt.float16)
            s2_t = consts.tile([P, mid_k], mybir.dt.float32)
            s4_t = consts.tile([P, out_k], mybir.dt.float32)
            bias_t = consts.tile([P, out_k], mybir.dt.float32)
            neg_half = consts.tile([P, 1], mybir.dt.float32)
            junk_w = consts.tile([P, 16], mybir.dt.float16)
            # mask built on-device (memsets) to keep the startup copy group
            # minimal; every copy<->transpose mode flip serializes the DMA
            # stream globally.
            for j in range(8):
                nc.vector.memset(mask_t[:, j : j + 1], 1 << (7 - j))
            nc.vector.memset(neg_half[:], -0.5)
            nc.vector.memset(junk_w[:], 0.25)

            # Warm the PE HAM clock gate with cheap junk matmuls while the
            # input pipeline fills; sized to drain just before the first real
            # matmul's operands land.
            warm_ps = psum.tile([P, 16], mybir.dt.float32, tag="warm")

            def warm_fill(n):
                for _ in range(n):
                    nc.tensor.matmul(
                        warm_ps[:16, :], junk_w[:], junk_w[:],
                        start=True, stop=True, skip_group_check=True,
                    )

            warm_fill(N_WARM)

            # Batched weight-byte loads: GB blocks per DMA so the copy/
            # transpose xbar-mode serialization fires once per group instead
            # of once per block.
            kb_bytes = {True: in_ // 8, False: mid // 8}

            def load_group(bp_d, g, k_blocks):
                kbb = k_blocks * P // 8
                byt = wpipe.tile(
                    [P, GB, kbb], mybir.dt.uint8, tag="bytes", bufs=2, name=f"byt{g}"
                )
                nc.gpsimd.dma_start(
                    byt[:],
                    bp_d[g * GB * P : (g + 1) * GB * P, :].rearrange(
                        "(j p) c -> p j c", p=P
                    ),
                )
                return byt

            def unpack_wT(byt, jg, m, k_blocks, scale_s0, fine=False):
                """Unpack 128-row weight block m (slot jg of its byte-group
                tile) into its transposed [P, k_blocks, P] fp16 tile. fine=True
                uses half-width chunks (shorter startup latency chain)."""
                u = uch // 2 if fine else uch
                sfx = "f" if fine else ""
                wT = wpipe.tile(
                    [P, k_blocks, P], mybir.dt.float16, tag="wT", bufs=5, name=f"wT{m}"
                )
                for c0 in range(0, k_blocks * P, u):
                    nb = u // 8
                    b0 = c0 // 8
                    masked = wpipe.tile(
                        [P, u], mybir.dt.uint8, tag=f"masked{sfx}", bufs=2,
                        name=f"mask{sfx}{m}_{c0}",
                    )
                    in0 = byt[:, jg, b0 : b0 + nb][:, :, None].broadcast_to([P, nb, 8])
                    in1 = mask_t[:][:, None, :].broadcast_to([P, nb, 8])
                    nc.vector.tensor_tensor(
                        masked[:].rearrange("p (b j) -> p b j", j=8),
                        in0,
                        in1,
                        mybir.AluOpType.bitwise_and,
                    )
                    wq = wpipe.tile(
                        [P, u], mybir.dt.float16, tag=f"wnat{sfx}",
                        bufs=2 if fine else 3,
                        name=f"wq{sfx}{m}_{c0}",
                    )
                    nc.scalar.activation(
                        wq[:], masked[:], Act.Sign, bias=neg_half[:, 0:1]
                    )
                    if scale_s0:
                        # DVE runs this at 2x (packed fp16) and its queue has
                        # no DMA copies to serialize behind, unlike GPSIMD
                        nc.vector.tensor_tensor(
                            wq[:], wq[:], s0r_t[:, c0 : c0 + u],
                            mybir.AluOpType.mult,
                        )
                    nc.sync.dma_start_transpose(
                        wT[:, c0 // P : (c0 + u) // P, :], wq[:]
                    )
                return wT

            # x.T in two batch halves, 4 k-bands each so the GEMM1 k-loop can
            # chase the DMA. All transposes share the sync queue (xbar mode).
            xH = [
                big.tile([P, in_k, fd], mybir.dt.float16, tag=f"xT{h}", name=f"xh{h}")
                for h in range(2)
            ]
            XB = 4
            kb = in_k // XB

            def x_band(h, band):
                nc.sync.dma_start_transpose(
                    xH[h][:, band * kb : (band + 1) * kb, :],
                    x_d[h * fd : (h + 1) * fd, band * kb * P : (band + 1) * kb * P],
                )

            hT = big.tile([P, mid_k, b], mybir.dt.float16, name="hT")

            def g1_pass(m, wT, c):
                ps = psum.tile([P, fd], mybir.dt.float32, tag="ps")
                for k in range(in_k):
                    nc.tensor.matmul(
                        ps[:],
                        wT[:, k, :],
                        xH[c][:, k, :],
                        start=(k == 0),
                        stop=(k == in_k - 1),
                    )

                nc.scalar.activation(
                    hT[:, m, c * fd : (c + 1) * fd],
                    ps[:],
                    Act.Copy,
                    scale=s2_t[:, m : m + 1],
                )

            # GEMM2 output staging: groups of blocks buffered in the (dead)
            # x-half SBUF slots, stored with one DMA per group; the final
            # group is kept small so the tail store is short.
            yT_v = yT_d.rearrange("(g p) c -> p g c", p=P)
            ygroups = []
            _o = 0
            while _o < out_k:
                rem = out_k - _o
                if rem > 8:
                    n = 8
                elif rem > 2:
                    n = rem - 2
                else:
                    n = 1
                ygroups.append((_o, n))
                _o += n
            o2group = {}
            for gi_, (gs, gn) in enumerate(ygroups):
                for oo in range(gs, gs + gn):
                    o2group[oo] = (gi_, gs, gn)
            yt_state = [None]

            def g2_pass(o, wT):
                gi_, gstart, glen = o2group[o]
                if o == gstart:
                    yt_state[0] = big.tile(
                        [P, glen, b], mybir.dt.float16,
                        tag=f"xT{gi_ % 2}", name=f"ytg{o}",
                    )
                yt_g = yt_state[0]
                for c in range(nbc):
                    ps = psum.tile([P, fd], mybir.dt.float32, tag="ps")
                    for k in range(mid_k):
                        nc.tensor.matmul(
                            ps[:],
                            wT[:, k, :],
                            hT[:, k, c * fd : (c + 1) * fd],
                            start=(k == 0),
                            stop=(k == mid_k - 1),
                        )
                    nc.scalar.activation(
                        yt_g[:, o - gstart, c * fd : (c + 1) * fd],
                        ps[:],
                        Act.Identity,
                        bias=bias_t[:, o : o + 1],
                        scale=s4_t[:, o : o + 1],
                    )
                if o == gstart + glen - 1:
                    nc.gpsimd.dma_start(yT_v[:, gstart : gstart + glen, :], yt_g[:])

            # Startup: first byte group + x half 0, unpack the first SB
            # blocks, x half 1; then the unified loop with unpack prefetched
            # SB blocks ahead (new byte groups loaded one group ahead).
            SB = 4
            n_blocks = mid_k + out_k
            n_groups = n_blocks // GB
            bgroups = [None] * n_groups

            def group_of(jj):
                g = jj // GB
                if bgroups[g] is None:
                    if g < mid_k // GB:
                        bgroups[g] = load_group(bp1_d, g, in_k)
                    else:
                        bgroups[g] = load_group(bp3_d, g - mid_k // GB, mid_k)
                return bgroups[g]

            def mk(jj):
                byt = group_of(jj)
                if jj + GB < n_blocks and (jj + 2) % GB == 0:
                    group_of(jj + GB)  # prefetch next byte group early
                if jj < mid_k:
                    return unpack_wT(byt, jj % GB, jj, in_k, True, fine=jj < 2)
                return unpack_wT(byt, jj % GB, jj, mid_k, False)

            # Startup: byte group 0 on gpsimd; consts copies lead the sync
            # queue (before its transposes — the copy<->transpose xbar-mode
            # serialization then fires once); x bands interleave with the
            # first blocks' unpack transposes so neither hogs the queue.
            group_of(0)
            for t, dd in (
                (s0r_t, s0r_d),
                (s2_t, s2_d),
                (s4_t, s4_d),
                (bias_t, bias_d),
            ):
                nc.sync.dma_start(t[:], dd[:])
            x_band(0, 0)
            pend = [mk(0)]
            for band in range(1, XB):
                x_band(0, band)
            pend.append(mk(1))
            x_band(1, 0)
            x_band(1, 1)
            pend.append(mk(2))
            x_band(1, 2)
            x_band(1, 3)
            pend.append(mk(3))

            for j in range(n_blocks):
                wT = pend.pop(0)
                if j < mid_k:
                    for c in range(nbc):
                        g1_pass(j, wT, c)
                else:
                    g2_pass(j - mid_k, wT)
                if j + SB < n_blocks:
                    pend.append(mk(j + SB))

    nc.compile()
    return nc


def make_in_maps(x, scaling0, bp1, scaling2, bp3, scaling4, bias, n_cores=N_CORES):
    b_full, in_ = x.shape
    mid = scaling2.shape[0]
    out = scaling4.shape[0]
    b = b_full // n_cores

    def pcol(v):
        return np.ascontiguousarray(v.astype(np.float32).reshape(-1, P).T)

    shared = {
        "bp1": np.ascontiguousarray(bp1.reshape(mid, in_ // 8).astype(np.uint8)),
        "bp3": np.ascontiguousarray(bp3.reshape(out, mid // 8).astype(np.uint8)),
        "s0rep": np.ascontiguousarray(
            np.broadcast_to(scaling0.astype(np.float16)[None, :], (P, in_))
        ),
        "s2": pcol(scaling2),
        "s4": pcol(scaling4),
        "bias": pcol(bias),
    }
    return [
        {"x": np.ascontiguousarray(x[c * b : (c + 1) * b]), **shared}
        for c in range(n_cores)
    ]


_PROGRAM_CACHE = {}


def run(x, scaling0, bp1, scaling2, bp3, scaling4, bias, **spmd_kwargs):
    """Compile (cached) + run on 8 cores; returns (y, BassKernelResults)."""
    if "nc" not in _PROGRAM_CACHE:
        _PROGRAM_CACHE["nc"] = build_program()
    nc = _PROGRAM_CACHE["nc"]
    in_maps = make_in_maps(x, scaling0, bp1, scaling2, bp3, scaling4, bias)
    res = run_bass_kernel_spmd(nc, in_maps, core_ids=list(range(N_CORES)), **spmd_kwargs)
    b = x.shape[0] // N_CORES
    y = np.empty((x.shape[0], scaling4.shape[0]), dtype=np.float16)
    for c in range(N_CORES):
        y[c * b : (c + 1) * b] = res.results[c]["yT"].T
    return y, res


def kernel(x, scaling0, bp1, scaling2, bp3, scaling4, bias):
    y, _ = run(x, scaling0, bp1, scaling2, bp3, scaling4, bias)
    return y
